# revision 29
# baseline (speedup 1.0000x reference)
"""Low-rank self-attention TRN2 kernel, tensor-parallel over heads on 8 cores.

Sharding: heads 2c,2c+1 on core c. Host merges low-rank factors (U@V) into
per-head effective QKV weights (same FLOPs as the sharded low-rank form since
rank==hidden/2), so each core computes its heads' q/k/v directly from the
full activations with zero collectives. o-proj is row-parallel (input-sharded
by head); partial outputs are reduced on host.

v4 schedule — fp8 DoubleRow for q/k projections and scores; bf16 for the
v path, P@V and o-proj (precision-critical: quantization noise on v/P/oT
passes straight through near-uniform softmax).

  1. q,k projections fused per chunk in fp8 DoubleRow (x fp8 streamed once,
     W pre-scaled fp8 on host; weight images column-ordered so each psum
     half-dh tile lands already "folded" into the [64-partition, 2-ktile]
     layout the DoubleRow scores need — the fold is a pure host-side
     permutation). x DMAs are pair-merged via a host layout and issued
     round-robin from 4 engines (dma_start costs the issuer ~500ns).
  2. v projection bf16 -> PE-transpose -> seq-major v_sm, streamed
     concurrently with early attention q-blocks.
  3. attention, qb outer / head inner; per (head, q-block of 1024):
       S.T tile [k:128, q:1024] = DoubleRow(kf, qf)
       P = exp(S.T * 2^-15)   ACT, PSUM -> SBUF bf16 (the only ACT work;
         ACT is the attention-phase critical path at ~1.04us/tile)
       denominator: DVE pair-add tree over the 32 P tiles (bf16 L0-L2,
         f32 L3-L4), then r = ones.T @ acc via f32r matmuls (1 cyc/row),
         reciprocal f32, broadcast via f32r matmul; evict oT2 = po*rbin
         on Pool.
       O.T[dh, q] += v_sm_kb @ P   (bf16)
     o-proj work for q-block N-1 is interleaved INTO q-block N's kb loop
     (one seq-tile per 4 kb) so its PE burst doesn't starve ACT.
Host: out = sum_c(partial_c) + o_b, partials in bf16.
"""

import math
import sys

sys.path.insert(0, "/opt/trn_rl_repo")

import numpy as np
import ml_dtypes

HIDDEN = 2048
HEADS = 16
DH = 128
S = 4096
NCORES = 8
HPC = HEADS // NCORES  # heads per core = 2
DPC = HPC * DH         # head dims per core = 256
QB = 1024              # q-block size in attention
BF16 = ml_dtypes.bfloat16
FP8 = ml_dtypes.float8_e4m3
SQ = 2.0 ** 9          # host scale on Wq (q stored as fp8 of q*SQ)
SK = 2.0 ** 6          # host scale on Wk
EXPSCALE = 1.0 / (SQ * SK)

_cache = {}


def build_nc(debug=False):
    import concourse.bacc as bacc
    import concourse.mybir as mybir
    import concourse.tile as tile
    from concourse import bass_isa
    from concourse.masks import make_identity

    dt = mybir.dt
    AF = mybir.ActivationFunctionType
    ALU = mybir.AluOpType
    DR = mybir.MatmulPerfMode.DoubleRow

    nc = bacc.Bacc(None, target_bir_lowering=False, debug=debug)
    # paired layouts: row (i2*128+p), col (chunk*1024 + t*512 + c) holds
    # xT[(i2*2+t)*128 + p, chunk*512 + c]
    xt8_d = nc.dram_tensor("xt8", [HIDDEN // 2, 2 * S], dt.float8e4,
                           kind="ExternalInput")
    xtb_d = nc.dram_tensor("xtb", [HIDDEN // 2, 2 * S], dt.bfloat16,
                           kind="ExternalInput")
    w8_ds = {
        p: nc.dram_tensor(f"w8{p}", [128, 8 * 2 * 256], dt.float8e4,
                          kind="ExternalInput")
        for p in "qk"
    }
    wv_d = nc.dram_tensor("wv", [128, 16 * 256], dt.bfloat16, kind="ExternalInput")
    wo_d = nc.dram_tensor("wo", [128, HPC * HIDDEN], dt.bfloat16,
                          kind="ExternalInput")
    out_d = nc.dram_tensor("out", [S, HIDDEN], dt.bfloat16,
                           kind="ExternalOutput")

    with tile.TileContext(nc) as tc:
        with tc.tile_pool(name="persist", bufs=1) as pp, \
             tc.tile_pool(name="xstr", bufs=12) as xp, \
             tc.tile_pool(name="pt", bufs=12) as ptp, \
             tc.tile_pool(name="trb", bufs=3) as trb, \
             tc.tile_pool(name="trf", bufs=3) as trf, \
             tc.tile_pool(name="rnorm", bufs=1) as rnp, \
             tc.tile_pool(name="outst", bufs=4) as osp, \
             tc.tile_pool(name="qkv_ps", bufs=2, space="PSUM") as qps, \
             tc.tile_pool(name="ps_s", bufs=2, space="PSUM") as pss, \
             tc.tile_pool(name="ps_o", bufs=1, space="PSUM") as pso:
            # ---- persistent tiles ----
            w8 = {}
            for p in "qk":
                w8[p] = pp.tile([128, 8, 2, 256], dt.float8e4, tag=f"w8{p}",
                                name=f"w8{p}")
                nc.sync.dma_start(out=w8[p][:], in_=w8_ds[p][:])
            wv_s = pp.tile([128, 16, 256], dt.bfloat16, tag="wv", name="wv_s")
            nc.sync.dma_start(out=wv_s[:], in_=wv_d[:])
            wo_s = pp.tile([128, HPC, HIDDEN], dt.bfloat16, tag="wo", name="wo_s")
            nc.sync.dma_start(out=wo_s[:], in_=wo_d[:])
            qf = pp.tile([128, 2, S], dt.float8e4, tag="qf", name="qf")
            kf = pp.tile([128, 2, S], dt.float8e4, tag="kf", name="kf")
            vT = {h: pp.tile([128, S], dt.bfloat16, tag=f"vT{h}", name=f"vT{h}")
                  for h in range(HPC)}
            v_sm = {h: pp.tile([128, S], dt.bfloat16, tag=f"vsm{h}", name=f"vsm{h}")
                    for h in range(HPC)}
            oT2 = pp.tile([128, HPC, S], dt.bfloat16, tag="oT2", name="oT2")
            ident = pp.tile([128, 128], dt.bfloat16, tag="ident", name="ident")
            make_identity(nc, ident[:])
            dma_engs = [nc.sync, nc.scalar, nc.gpsimd]
            dma_rr = [0]

            def dma(out, in_, engs=None):
                engs = engs or dma_engs
                eng = engs[dma_rr[0] % len(engs)]
                dma_rr[0] += 1
                eng.dma_start(out=out, in_=in_)

            # ---- Stage 1a: q,k projections fused, fp8 DoubleRow ----
            # x8 streamed once per chunk; q psum in pss (double-buffered
            # [128,1024], d-halves in columns), k psum in pso.
            for chunk in range(8):
                base = chunk * 512
                ps_q = pss.tile([128, 1024], dt.float32, tag="pss",
                                name=f"psq_{chunk}")
                ps_k = pso.tile([128, 1024], dt.float32, tag="pso",
                                name=f"psk_{chunk}")
                x8ts = []
                for i2 in range(8):
                    x8t = xp.tile([128, 2, 512], dt.float8e4, tag="x8",
                                  name=f"x8_{chunk}_{i2}")
                    dma(x8t[:], xt8_d[i2 * 128:(i2 + 1) * 128,
                                      chunk * 1024:(chunk + 1) * 1024])
                    x8ts.append(x8t)
                for i2 in range(8):
                    for d in range(2):
                        nc.tensor.matmul(
                            ps_q[:, d * 512:(d + 1) * 512],
                            w8["q"][:, i2, :, d * 128:(d + 1) * 128],
                            x8ts[i2][:],
                            start=(i2 == 0),
                            stop=(i2 == 7),
                            perf_mode=DR,
                            skip_group_check=True,
                        )
                for i2 in range(8):
                    for d in range(2):
                        nc.tensor.matmul(
                            ps_k[:, d * 512:(d + 1) * 512],
                            w8["k"][:, i2, :, d * 128:(d + 1) * 128],
                            x8ts[i2][:],
                            start=(i2 == 0),
                            stop=(i2 == 7),
                            perf_mode=DR,
                            skip_group_check=True,
                        )
                for d in range(2):
                    nc.vector.tensor_copy(qf[:, d, base:base + 512],
                                          ps_q[:, d * 512:(d + 1) * 512])
                    nc.vector.tensor_copy(kf[:, d, base:base + 512],
                                          ps_k[:, d * 512:(d + 1) * 512])

            # ---- Stage 1b: v projection (bf16) + transpose to seq-major.
            # Chunks 0-3 inline (qps/pso psums only -- pss stays free so
            # attention's score psums are never blocked); chunks 4-7 are
            # emitted inside the first attention block's kb loop.
            vdma_tiles = {}

            def emit_vchunk_dma(chunk, engs):
                tiles = []
                for i2 in range(8):
                    xbt = xp.tile([128, 2, 512], dt.bfloat16, tag="xb",
                                  name=f"xb_{chunk}_{i2}")
                    dma(xbt[:], xtb_d[i2 * 128:(i2 + 1) * 128,
                                      chunk * 1024:(chunk + 1) * 1024],
                        engs=engs)
                    tiles.append(xbt)
                vdma_tiles[chunk] = tiles

            def emit_vchunk_mm(chunk, tpools):
                base = chunk * 512
                ps_h = [qps.tile([128, 512], dt.float32, tag="rps",
                                 name=f"psv_{chunk}_{h}")
                        for h in range(HPC)]
                for i2 in range(8):
                    xbt = vdma_tiles[chunk][i2]
                    for t in range(2):
                        for h in range(HPC):
                            nc.tensor.matmul(
                                ps_h[h][:],
                                wv_s[:, i2 * 2 + t, h * 128:(h + 1) * 128],
                                xbt[:, t, :],
                                start=(i2 == 0 and t == 0),
                                stop=(i2 == 7 and t == 1),
                            )
                for h in range(HPC):
                    nc.vector.tensor_copy(vT[h][:, base:base + 512], ps_h[h][:])
                for h in range(HPC):
                    for jj in range(4):
                        j = chunk * 4 + jj
                        tp_t = tpools[jj % len(tpools)].tile(
                            [128, 128], dt.bfloat16, tag="rps" if True else "",
                            name=f"vt_{h}_{j}")
                        nc.tensor.transpose(
                            tp_t[:], vT[h][:, j * 128:(j + 1) * 128], ident[:]
                        )
                        nc.vector.tensor_copy(
                            v_sm[h][:, j * 128:(j + 1) * 128], tp_t[:]
                        )

            for chunk in range(4):
                emit_vchunk_dma(chunk, [nc.sync, nc.gpsimd])
                emit_vchunk_mm(chunk, [qps])
            # prefetch xb for chunks 4,5 (6,7 follow inside attention)
            emit_vchunk_dma(4, [nc.sync, nc.gpsimd])
            emit_vchunk_dma(5, [nc.sync, nc.gpsimd])

            # ---- Stage 2: attention; o-proj of earlier q-blocks
            # interleaved one (t,nb) tile per kb ----
            oproj_work = []  # (qb, t, nb)

            def emit_oproj(t, nb):
                ps = qps.tile([128, 512], dt.float32, tag="rps",
                              name=f"ops_{t}_{nb}")
                for h in range(HPC):
                    nc.tensor.matmul(
                        ps[:],
                        oT2[:, h, t * 128:(t + 1) * 128],
                        wo_s[:, h, nb * 512:(nb + 1) * 512],
                        start=(h == 0),
                        stop=(h == HPC - 1),
                    )
                ot_ = osp.tile([128, 512], dt.bfloat16, tag="outst",
                               name=f"ot_{t}_{nb}")
                nc.vector.tensor_copy(ot_[:], ps[:])
                dma(out_d[t * 128:(t + 1) * 128, nb * 512:(nb + 1) * 512],
                    ot_[:], engs=[nc.sync, nc.gpsimd])


            def finish_block(qb, h, po, acc):
                # PE-free normalization: cross-partition sum on Pool,
                # reciprocal + scale on DVE.
                rsum = rnp.tile([128, QB], dt.float32, tag="rsum",
                                name=f"rsum_{qb}_{h}")
                nc.gpsimd.partition_all_reduce(rsum[:], acc[:], 128,
                                               bass_isa.ReduceOp.add)
                rinv = rnp.tile([128, QB], dt.float32, tag="rinv",
                                name=f"rinv_{qb}_{h}")
                nc.vector.reciprocal(rinv[:], rsum[:])
                nc.vector.tensor_tensor(
                    oT2[:, h, qb * QB:(qb + 1) * QB], po[:], rinv[:], ALU.mult,
                )
                if h == HPC - 1:
                    for t in range(qb * (QB // 128), (qb + 1) * (QB // 128)):
                        for nb in range(HIDDEN // 512):
                            oproj_work.append((t, nb))

            for qb in range(S // QB):
                for h in range(HPC):
                    first_block = (qb == 0 and h == 0)
                    po = pso.tile([128, QB], dt.float32, tag="pso",
                                  name=f"po_{qb}_{h}")
                    s1 = []
                    s2 = []
                    s3 = []
                    s4 = []
                    acc = None
                    for kb in range(S // 128):
                        ps = pss.tile([128, QB], dt.float32, tag="pss",
                                      name=f"ps_{qb}_{h}_{kb}")
                        for j in range(QB // 512):
                            nc.tensor.matmul(
                                ps[:, j * 512:(j + 1) * 512],
                                kf[h * 64:(h + 1) * 64, :, kb * 128:(kb + 1) * 128],
                                qf[h * 64:(h + 1) * 64, :,
                                   qb * QB + j * 512:qb * QB + (j + 1) * 512],
                                start=True,
                                stop=True,
                                perf_mode=DR,
                            )
                        pt = ptp.tile([128, QB], dt.bfloat16, tag="pt",
                                      name=f"pt_{qb}_{h}_{kb}")
                        nc.scalar.activation(pt[:], ps[:], AF.Exp, scale=EXPSCALE)
                        for j in range(QB // 512):
                            nc.tensor.matmul(
                                po[:, j * 512:(j + 1) * 512],
                                v_sm[h][:, kb * 128:(kb + 1) * 128],
                                pt[:, j * 512:(j + 1) * 512],
                                start=(kb == 0),
                                stop=(kb == 31),
                                skip_group_check=True,
                            )
                        # first block: stream v chunks 4-7 through spare slots
                        if first_block:
                            if kb == 5:
                                emit_vchunk_dma(6, [nc.sync, nc.gpsimd])
                            elif kb == 11:
                                emit_vchunk_dma(7, [nc.sync, nc.gpsimd])
                            if kb in (3, 9, 15, 21):
                                emit_vchunk_mm(4 + (kb - 3) // 6, [qps])
                        if kb % 2 == 1 and oproj_work:
                            emit_oproj(*oproj_work.pop(0))
                        # denominator pair-add tree on DVE
                        s1.append(pt)
                        if len(s1) == 2:
                            a, b = s1
                            o = trb.tile([128, QB], dt.bfloat16, tag="s1",
                                         name=f"s1_{qb}_{h}_{kb}")
                            nc.vector.tensor_tensor(o[:], a[:], b[:], ALU.add)
                            s1 = []
                            s2.append(o)
                        if len(s2) == 2:
                            a, b = s2
                            o = trb.tile([128, QB], dt.bfloat16, tag="s2",
                                         name=f"s2_{qb}_{h}_{kb}")
                            nc.vector.tensor_tensor(o[:], a[:], b[:], ALU.add)
                            s2 = []
                            s3.append(o)
                        if len(s3) == 2:
                            a, b = s3
                            s3 = []
                            acc2 = trf.tile([128, QB], dt.float32, tag="trf",
                                            name=f"acc_{qb}_{h}_{kb}")
                            if acc is None:
                                nc.vector.tensor_tensor(acc2[:], a[:], b[:],
                                                        ALU.add)
                            else:
                                o = trb.tile([128, QB], dt.bfloat16, tag="s3",
                                             name=f"s3_{qb}_{h}_{kb}")
                                nc.vector.tensor_tensor(o[:], a[:], b[:],
                                                        ALU.add)
                                nc.vector.tensor_tensor(acc2[:], acc[:], o[:],
                                                        ALU.add)
                            acc = acc2
                    finish_block(qb, h, po, acc)
            # drain remaining o-proj work (last q-block)
            for item in oproj_work:
                emit_oproj(*item)
    nc.finalize()
    return nc


def host_prep(hidden_states, q_V, q_U, k_V, k_U, v_V, v_U, o_W):
    """Build per-core input maps (host-side sharding + layout)."""
    x = np.asarray(hidden_states, np.float32).reshape(S, HIDDEN)
    xT = np.ascontiguousarray(x.T)

    def paired(xt):  # [HIDDEN, S] -> [HIDDEN/2, 2S] pair-merged DMA layout
        A = xt.reshape(8, 2, 128, 8, 512)        # [i2, t, p, chunk, c]
        A = A.transpose(0, 2, 3, 1, 4)           # [i2, p, chunk, t, c]
        return np.ascontiguousarray(A.reshape(HIDDEN // 2, 2 * S))

    xT8 = paired(xT).astype(FP8)
    xTb = paired(xT).astype(BF16)
    Wq = (np.asarray(q_U, np.float32) @ np.asarray(q_V, np.float32)) \
        / math.sqrt(DH) * SQ
    Wk = np.asarray(k_U, np.float32) @ np.asarray(k_V, np.float32) * SK
    Wv = np.asarray(v_U, np.float32) @ np.asarray(v_V, np.float32)
    oW = np.asarray(o_W, np.float32)

    def w8_image(WT):
        # [HIDDEN, DPC] -> [128, 8*2*2*128] fp8 image with folded column
        # order: free idx = i2*512 + t*256 + d*128 + h*64 + j, selecting
        # WT[(i2*2+t)*128 + p, h*128 + d*64 + j].
        A = WT.reshape(8, 2, 128, HPC, 2, 64)   # [i2, t, p, h, d, j]
        A = A.transpose(2, 0, 1, 4, 3, 5)       # [p, i2, t, d, h, j]
        return np.ascontiguousarray(A.reshape(128, 8 * 2 * 2 * 128)).astype(FP8)

    def wv_image(WT):  # [HIDDEN, DPC] -> [128, 16*DPC] sbuf image
        return np.ascontiguousarray(
            WT.reshape(16, 128, DPC).transpose(1, 0, 2).reshape(128, 16 * DPC)
        ).astype(BF16)

    def wo_image(oWcT):  # [DPC, HIDDEN] -> [128, HPC*HIDDEN]
        return np.ascontiguousarray(
            oWcT.reshape(HPC, 128, HIDDEN).transpose(1, 0, 2).reshape(128, HPC * HIDDEN)
        ).astype(BF16)

    in_maps = []
    for c in range(NCORES):
        sl = slice(c * DPC, (c + 1) * DPC)
        in_maps.append({
            "xt8": xT8,
            "xtb": xTb,
            "w8q": w8_image(np.ascontiguousarray(Wq[sl, :].T)),
            "w8k": w8_image(np.ascontiguousarray(Wk[sl, :].T)),
            "wv": wv_image(np.ascontiguousarray(Wv[sl, :].T)),
            "wo": wo_image(np.ascontiguousarray(oW[:, sl].T)),
        })
    return in_maps


def run(inputs, trace=False, tmpdir=None):
    from concourse.bass_utils import run_bass_kernel_spmd

    if "nc" not in _cache:
        _cache["nc"] = build_nc()
    nc = _cache["nc"]
    in_maps = host_prep(
        inputs["hidden_states"], inputs["q_V"], inputs["q_U"], inputs["k_V"],
        inputs["k_U"], inputs["v_V"], inputs["v_U"], inputs["o_W"],
    )
    res = run_bass_kernel_spmd(
        nc, in_maps, core_ids=list(range(NCORES)), trace=trace, tmpdir=tmpdir
    )
    acc = np.zeros((S, HIDDEN), np.float64)
    for c in range(NCORES):
        acc += res.results[c]["out"].astype(np.float64)
    out = (acc + np.asarray(inputs["o_b"], np.float64)[None, :]).astype(np.float32)
    return out.reshape(1, S, HIDDEN), res


def kernel(**inputs) -> np.ndarray:
    out, _ = run(inputs, trace=False)
    return out


# revision 30
# speedup vs baseline: 1.0061x; 1.0061x over previous
"""Low-rank self-attention TRN2 kernel, tensor-parallel over heads on 8 cores.

Sharding: heads 2c,2c+1 on core c. Host merges low-rank factors (U@V) into
per-head effective QKV weights (same FLOPs as the sharded low-rank form since
rank==hidden/2), so each core computes its heads' q/k/v directly from the
full activations with zero collectives. o-proj is row-parallel (input-sharded
by head); partial outputs are reduced on host.

v4 schedule — fp8 DoubleRow for q/k projections and scores; bf16 for the
v path, P@V and o-proj (precision-critical: quantization noise on v/P/oT
passes straight through near-uniform softmax).

  1. q,k projections fused per chunk in fp8 DoubleRow (x fp8 streamed once,
     W pre-scaled fp8 on host; weight images column-ordered so each psum
     half-dh tile lands already "folded" into the [64-partition, 2-ktile]
     layout the DoubleRow scores need — the fold is a pure host-side
     permutation). x DMAs are pair-merged via a host layout and issued
     round-robin from 4 engines (dma_start costs the issuer ~500ns).
  2. v projection bf16 -> PE-transpose -> seq-major v_sm, streamed
     concurrently with early attention q-blocks.
  3. attention, qb outer / head inner; per (head, q-block of 1024):
       S.T tile [k:128, q:1024] = DoubleRow(kf, qf)
       P = exp(S.T * 2^-15)   ACT, PSUM -> SBUF bf16 (the only ACT work;
         ACT is the attention-phase critical path at ~1.04us/tile)
       denominator: DVE pair-add tree over the 32 P tiles (bf16 L0-L2,
         f32 L3-L4), then r = ones.T @ acc via f32r matmuls (1 cyc/row),
         reciprocal f32, broadcast via f32r matmul; evict oT2 = po*rbin
         on Pool.
       O.T[dh, q] += v_sm_kb @ P   (bf16)
     o-proj work for q-block N-1 is interleaved INTO q-block N's kb loop
     (one seq-tile per 4 kb) so its PE burst doesn't starve ACT.
Host: out = sum_c(partial_c) + o_b, partials in bf16.
"""

import math
import sys

sys.path.insert(0, "/opt/trn_rl_repo")

import numpy as np
import ml_dtypes

HIDDEN = 2048
HEADS = 16
DH = 128
S = 4096
NCORES = 8
HPC = HEADS // NCORES  # heads per core = 2
DPC = HPC * DH         # head dims per core = 256
QB = 1024              # q-block size in attention
BF16 = ml_dtypes.bfloat16
FP8 = ml_dtypes.float8_e4m3
SQ = 2.0 ** 9          # host scale on Wq (q stored as fp8 of q*SQ)
SK = 2.0 ** 6          # host scale on Wk
EXPSCALE = 1.0 / (SQ * SK)

_cache = {}


def build_nc(debug=False):
    import concourse.bacc as bacc
    import concourse.mybir as mybir
    import concourse.tile as tile
    from concourse import bass_isa
    from concourse.masks import make_identity

    dt = mybir.dt
    AF = mybir.ActivationFunctionType
    ALU = mybir.AluOpType
    DR = mybir.MatmulPerfMode.DoubleRow

    nc = bacc.Bacc(None, target_bir_lowering=False, debug=debug)
    # paired layouts: row (i2*128+p), col (chunk*1024 + t*512 + c) holds
    # xT[(i2*2+t)*128 + p, chunk*512 + c]
    xt8_d = nc.dram_tensor("xt8", [HIDDEN // 2, 2 * S], dt.float8e4,
                           kind="ExternalInput")
    xtb_d = nc.dram_tensor("xtb", [HIDDEN // 2, 2 * S], dt.bfloat16,
                           kind="ExternalInput")
    w8_ds = {
        p: nc.dram_tensor(f"w8{p}", [128, 8 * 2 * 256], dt.float8e4,
                          kind="ExternalInput")
        for p in "qk"
    }
    wv_d = nc.dram_tensor("wv", [128, 16 * 256], dt.bfloat16, kind="ExternalInput")
    wo_d = nc.dram_tensor("wo", [128, HPC * HIDDEN], dt.bfloat16,
                          kind="ExternalInput")
    out_d = nc.dram_tensor("out", [S, HIDDEN], dt.bfloat16,
                           kind="ExternalOutput")

    with tile.TileContext(nc) as tc:
        with tc.tile_pool(name="persist", bufs=1) as pp, \
             tc.tile_pool(name="xstr", bufs=12) as xp, \
             tc.tile_pool(name="pt", bufs=12) as ptp, \
             tc.tile_pool(name="trb", bufs=3) as trb, \
             tc.tile_pool(name="trf", bufs=3) as trf, \
             tc.tile_pool(name="rnorm", bufs=1) as rnp, \
             tc.tile_pool(name="outst", bufs=4) as osp, \
             tc.tile_pool(name="qkv_ps", bufs=2, space="PSUM") as qps, \
             tc.tile_pool(name="ps_s", bufs=2, space="PSUM") as pss, \
             tc.tile_pool(name="ps_o", bufs=1, space="PSUM") as pso:
            # ---- persistent tiles ----
            w8 = {}
            for p in "qk":
                w8[p] = pp.tile([128, 8, 2, 256], dt.float8e4, tag=f"w8{p}",
                                name=f"w8{p}")
                nc.sync.dma_start(out=w8[p][:], in_=w8_ds[p][:])
            wv_s = pp.tile([128, 16, 256], dt.bfloat16, tag="wv", name="wv_s")
            nc.sync.dma_start(out=wv_s[:], in_=wv_d[:])
            wo_s = pp.tile([128, HPC, HIDDEN], dt.bfloat16, tag="wo", name="wo_s")
            nc.sync.dma_start(out=wo_s[:], in_=wo_d[:])
            qf = pp.tile([128, 2, S], dt.float8e4, tag="qf", name="qf")
            kf = pp.tile([128, 2, S], dt.float8e4, tag="kf", name="kf")
            vT = {h: pp.tile([128, S], dt.bfloat16, tag=f"vT{h}", name=f"vT{h}")
                  for h in range(HPC)}
            v_sm = {h: pp.tile([128, S], dt.bfloat16, tag=f"vsm{h}", name=f"vsm{h}")
                    for h in range(HPC)}
            oT2 = pp.tile([128, HPC, S], dt.bfloat16, tag="oT2", name="oT2")
            ident = pp.tile([128, 128], dt.bfloat16, tag="ident", name="ident")
            make_identity(nc, ident[:])
            dma_engs = [nc.sync, nc.scalar, nc.gpsimd]
            dma_rr = [0]

            def dma(out, in_, engs=None):
                engs = engs or dma_engs
                eng = engs[dma_rr[0] % len(engs)]
                dma_rr[0] += 1
                eng.dma_start(out=out, in_=in_)

            # ---- Stage 1a: q,k projections fused, fp8 DoubleRow ----
            # x8 streamed once per chunk; q psum in pss (double-buffered
            # [128,1024], d-halves in columns), k psum in pso.
            for chunk in range(8):
                base = chunk * 512
                ps_q = pss.tile([128, 1024], dt.float32, tag="pss",
                                name=f"psq_{chunk}")
                ps_k = pso.tile([128, 1024], dt.float32, tag="pso",
                                name=f"psk_{chunk}")
                x8ts = []
                for i2 in range(8):
                    x8t = xp.tile([128, 2, 512], dt.float8e4, tag="x8",
                                  name=f"x8_{chunk}_{i2}")
                    dma(x8t[:], xt8_d[i2 * 128:(i2 + 1) * 128,
                                      chunk * 1024:(chunk + 1) * 1024])
                    x8ts.append(x8t)
                for i2 in range(8):
                    for d in range(2):
                        nc.tensor.matmul(
                            ps_q[:, d * 512:(d + 1) * 512],
                            w8["q"][:, i2, :, d * 128:(d + 1) * 128],
                            x8ts[i2][:],
                            start=(i2 == 0),
                            stop=(i2 == 7),
                            perf_mode=DR,
                            skip_group_check=True,
                        )
                for i2 in range(8):
                    for d in range(2):
                        nc.tensor.matmul(
                            ps_k[:, d * 512:(d + 1) * 512],
                            w8["k"][:, i2, :, d * 128:(d + 1) * 128],
                            x8ts[i2][:],
                            start=(i2 == 0),
                            stop=(i2 == 7),
                            perf_mode=DR,
                            skip_group_check=True,
                        )
                for d in range(2):
                    nc.vector.tensor_copy(qf[:, d, base:base + 512],
                                          ps_q[:, d * 512:(d + 1) * 512])
                    nc.vector.tensor_copy(kf[:, d, base:base + 512],
                                          ps_k[:, d * 512:(d + 1) * 512])

            # ---- Stage 1b: v projection (bf16) + transpose to seq-major.
            # Chunks 0-3 inline (qps/pso psums only -- pss stays free so
            # attention's score psums are never blocked); chunks 4-7 are
            # emitted inside the first attention block's kb loop.
            vdma_tiles = {}

            def emit_vchunk_dma(chunk, engs):
                tiles = []
                for i2 in range(8):
                    xbt = xp.tile([128, 2, 512], dt.bfloat16, tag="xb",
                                  name=f"xb_{chunk}_{i2}")
                    dma(xbt[:], xtb_d[i2 * 128:(i2 + 1) * 128,
                                      chunk * 1024:(chunk + 1) * 1024],
                        engs=engs)
                    tiles.append(xbt)
                vdma_tiles[chunk] = tiles

            def emit_vchunk_mm(chunk, tpools):
                base = chunk * 512
                ps_h = [qps.tile([128, 512], dt.float32, tag="rps",
                                 name=f"psv_{chunk}_{h}")
                        for h in range(HPC)]
                for i2 in range(8):
                    xbt = vdma_tiles[chunk][i2]
                    for t in range(2):
                        for h in range(HPC):
                            nc.tensor.matmul(
                                ps_h[h][:],
                                wv_s[:, i2 * 2 + t, h * 128:(h + 1) * 128],
                                xbt[:, t, :],
                                start=(i2 == 0 and t == 0),
                                stop=(i2 == 7 and t == 1),
                            )
                for h in range(HPC):
                    nc.vector.tensor_copy(vT[h][:, base:base + 512], ps_h[h][:])
                for h in range(HPC):
                    for jj in range(4):
                        j = chunk * 4 + jj
                        tp_t = tpools[jj % len(tpools)].tile(
                            [128, 128], dt.bfloat16, tag="rps" if True else "",
                            name=f"vt_{h}_{j}")
                        nc.tensor.transpose(
                            tp_t[:], vT[h][:, j * 128:(j + 1) * 128], ident[:]
                        )
                        nc.vector.tensor_copy(
                            v_sm[h][:, j * 128:(j + 1) * 128], tp_t[:]
                        )

            for chunk in range(4):
                emit_vchunk_dma(chunk, [nc.sync, nc.gpsimd])
                emit_vchunk_mm(chunk, [qps])
            # prefetch xb for chunks 4,5 (6,7 follow inside attention)
            emit_vchunk_dma(4, [nc.sync, nc.gpsimd])
            emit_vchunk_dma(5, [nc.sync, nc.gpsimd])

            # ---- Stage 2: attention; o-proj of earlier q-blocks
            # interleaved one (t,nb) tile per kb ----
            oproj_work = []  # (qb, t, nb)

            def emit_oproj(t, nb):
                ps = qps.tile([128, 512], dt.float32, tag="rps",
                              name=f"ops_{t}_{nb}")
                for h in range(HPC):
                    nc.tensor.matmul(
                        ps[:],
                        oT2[:, h, t * 128:(t + 1) * 128],
                        wo_s[:, h, nb * 512:(nb + 1) * 512],
                        start=(h == 0),
                        stop=(h == HPC - 1),
                    )
                ot_ = osp.tile([128, 512], dt.bfloat16, tag="outst",
                               name=f"ot_{t}_{nb}")
                nc.vector.tensor_copy(ot_[:], ps[:])
                dma(out_d[t * 128:(t + 1) * 128, nb * 512:(nb + 1) * 512],
                    ot_[:], engs=[nc.sync, nc.gpsimd])


            def finish_block(qb, h, po, acc):
                # PE-free normalization: cross-partition sum on Pool,
                # reciprocal + scale on DVE.
                rsum = rnp.tile([128, QB], dt.float32, tag="rsum",
                                name=f"rsum_{qb}_{h}")
                nc.gpsimd.partition_all_reduce(rsum[:], acc[:], 128,
                                               bass_isa.ReduceOp.add)
                rinv = rnp.tile([128, QB], dt.float32, tag="rinv",
                                name=f"rinv_{qb}_{h}")
                nc.vector.reciprocal(rinv[:], rsum[:])
                nc.vector.tensor_tensor(
                    oT2[:, h, qb * QB:(qb + 1) * QB], po[:], rinv[:], ALU.mult,
                )
                if h == HPC - 1:
                    for t in range(qb * (QB // 128), (qb + 1) * (QB // 128)):
                        for nb in range(HIDDEN // 512):
                            oproj_work.append((t, nb))

            for qb in range(S // QB):
                for h in range(HPC):
                    first_block = (qb == 0 and h == 0)
                    po = pso.tile([128, QB], dt.float32, tag="pso",
                                  name=f"po_{qb}_{h}")
                    s1 = []
                    s2 = []
                    s3 = []
                    s4 = []
                    acc = None
                    for kb in range(S // 128):
                        ps = pss.tile([128, QB], dt.float32, tag="pss",
                                      name=f"ps_{qb}_{h}_{kb}")
                        for j in range(QB // 512):
                            nc.tensor.matmul(
                                ps[:, j * 512:(j + 1) * 512],
                                kf[h * 64:(h + 1) * 64, :, kb * 128:(kb + 1) * 128],
                                qf[h * 64:(h + 1) * 64, :,
                                   qb * QB + j * 512:qb * QB + (j + 1) * 512],
                                start=True,
                                stop=True,
                                perf_mode=DR,
                            )
                        pt = ptp.tile([128, QB], dt.bfloat16, tag="pt",
                                      name=f"pt_{qb}_{h}_{kb}")
                        nc.scalar.activation(pt[:], ps[:], AF.Exp, scale=EXPSCALE)
                        for j in range(QB // 512):
                            nc.tensor.matmul(
                                po[:, j * 512:(j + 1) * 512],
                                v_sm[h][:, kb * 128:(kb + 1) * 128],
                                pt[:, j * 512:(j + 1) * 512],
                                start=(kb == 0),
                                stop=(kb == 31),
                                skip_group_check=True,
                            )
                        # first block: stream v chunks 4-7 through spare slots
                        if first_block:
                            if kb == 5:
                                emit_vchunk_dma(6, [nc.sync, nc.gpsimd])
                            elif kb == 11:
                                emit_vchunk_dma(7, [nc.sync, nc.gpsimd])
                            if kb in (3, 9, 15, 21):
                                emit_vchunk_mm(4 + (kb - 3) // 6, [qps])
                        if kb % 2 == 1 and kb >= 5 and oproj_work:
                            emit_oproj(*oproj_work.pop(0))
                        if kb in (21, 23, 25, 27) and oproj_work:
                            emit_oproj(*oproj_work.pop(0))
                        # denominator pair-add tree on DVE
                        s1.append(pt)
                        if len(s1) == 2:
                            a, b = s1
                            o = trb.tile([128, QB], dt.bfloat16, tag="s1",
                                         name=f"s1_{qb}_{h}_{kb}")
                            nc.vector.tensor_tensor(o[:], a[:], b[:], ALU.add)
                            s1 = []
                            s2.append(o)
                        if len(s2) == 2:
                            a, b = s2
                            o = trb.tile([128, QB], dt.bfloat16, tag="s2",
                                         name=f"s2_{qb}_{h}_{kb}")
                            nc.vector.tensor_tensor(o[:], a[:], b[:], ALU.add)
                            s2 = []
                            s3.append(o)
                        if len(s3) == 2:
                            a, b = s3
                            s3 = []
                            acc2 = trf.tile([128, QB], dt.float32, tag="trf",
                                            name=f"acc_{qb}_{h}_{kb}")
                            if acc is None:
                                nc.vector.tensor_tensor(acc2[:], a[:], b[:],
                                                        ALU.add)
                            else:
                                o = trb.tile([128, QB], dt.bfloat16, tag="s3",
                                             name=f"s3_{qb}_{h}_{kb}")
                                nc.vector.tensor_tensor(o[:], a[:], b[:],
                                                        ALU.add)
                                nc.vector.tensor_tensor(acc2[:], acc[:], o[:],
                                                        ALU.add)
                            acc = acc2
                    finish_block(qb, h, po, acc)
            # drain remaining o-proj work (last q-block)
            for item in oproj_work:
                emit_oproj(*item)
    nc.finalize()
    return nc


def host_prep(hidden_states, q_V, q_U, k_V, k_U, v_V, v_U, o_W):
    """Build per-core input maps (host-side sharding + layout)."""
    x = np.asarray(hidden_states, np.float32).reshape(S, HIDDEN)
    xT = np.ascontiguousarray(x.T)

    def paired(xt):  # [HIDDEN, S] -> [HIDDEN/2, 2S] pair-merged DMA layout
        A = xt.reshape(8, 2, 128, 8, 512)        # [i2, t, p, chunk, c]
        A = A.transpose(0, 2, 3, 1, 4)           # [i2, p, chunk, t, c]
        return np.ascontiguousarray(A.reshape(HIDDEN // 2, 2 * S))

    xT8 = paired(xT).astype(FP8)
    xTb = paired(xT).astype(BF16)
    Wq = (np.asarray(q_U, np.float32) @ np.asarray(q_V, np.float32)) \
        / math.sqrt(DH) * SQ
    Wk = np.asarray(k_U, np.float32) @ np.asarray(k_V, np.float32) * SK
    Wv = np.asarray(v_U, np.float32) @ np.asarray(v_V, np.float32)
    oW = np.asarray(o_W, np.float32)

    def w8_image(WT):
        # [HIDDEN, DPC] -> [128, 8*2*2*128] fp8 image with folded column
        # order: free idx = i2*512 + t*256 + d*128 + h*64 + j, selecting
        # WT[(i2*2+t)*128 + p, h*128 + d*64 + j].
        A = WT.reshape(8, 2, 128, HPC, 2, 64)   # [i2, t, p, h, d, j]
        A = A.transpose(2, 0, 1, 4, 3, 5)       # [p, i2, t, d, h, j]
        return np.ascontiguousarray(A.reshape(128, 8 * 2 * 2 * 128)).astype(FP8)

    def wv_image(WT):  # [HIDDEN, DPC] -> [128, 16*DPC] sbuf image
        return np.ascontiguousarray(
            WT.reshape(16, 128, DPC).transpose(1, 0, 2).reshape(128, 16 * DPC)
        ).astype(BF16)

    def wo_image(oWcT):  # [DPC, HIDDEN] -> [128, HPC*HIDDEN]
        return np.ascontiguousarray(
            oWcT.reshape(HPC, 128, HIDDEN).transpose(1, 0, 2).reshape(128, HPC * HIDDEN)
        ).astype(BF16)

    in_maps = []
    for c in range(NCORES):
        sl = slice(c * DPC, (c + 1) * DPC)
        in_maps.append({
            "xt8": xT8,
            "xtb": xTb,
            "w8q": w8_image(np.ascontiguousarray(Wq[sl, :].T)),
            "w8k": w8_image(np.ascontiguousarray(Wk[sl, :].T)),
            "wv": wv_image(np.ascontiguousarray(Wv[sl, :].T)),
            "wo": wo_image(np.ascontiguousarray(oW[:, sl].T)),
        })
    return in_maps


def run(inputs, trace=False, tmpdir=None):
    from concourse.bass_utils import run_bass_kernel_spmd

    if "nc" not in _cache:
        _cache["nc"] = build_nc()
    nc = _cache["nc"]
    in_maps = host_prep(
        inputs["hidden_states"], inputs["q_V"], inputs["q_U"], inputs["k_V"],
        inputs["k_U"], inputs["v_V"], inputs["v_U"], inputs["o_W"],
    )
    res = run_bass_kernel_spmd(
        nc, in_maps, core_ids=list(range(NCORES)), trace=trace, tmpdir=tmpdir
    )
    acc = np.zeros((S, HIDDEN), np.float64)
    for c in range(NCORES):
        acc += res.results[c]["out"].astype(np.float64)
    out = (acc + np.asarray(inputs["o_b"], np.float64)[None, :]).astype(np.float32)
    return out.reshape(1, S, HIDDEN), res


def kernel(**inputs) -> np.ndarray:
    out, _ = run(inputs, trace=False)
    return out


# revision 31
# speedup vs baseline: 1.0075x; 1.0014x over previous
"""Low-rank self-attention TRN2 kernel, tensor-parallel over heads on 8 cores.

Sharding: heads 2c,2c+1 on core c. Host merges low-rank factors (U@V) into
per-head effective QKV weights (same FLOPs as the sharded low-rank form since
rank==hidden/2), so each core computes its heads' q/k/v directly from the
full activations with zero collectives. o-proj is row-parallel (input-sharded
by head); partial outputs are reduced on host.

v4 schedule — fp8 DoubleRow for q/k projections and scores; bf16 for the
v path, P@V and o-proj (precision-critical: quantization noise on v/P/oT
passes straight through near-uniform softmax).

  1. q,k projections fused per chunk in fp8 DoubleRow (x fp8 streamed once,
     W pre-scaled fp8 on host; weight images column-ordered so each psum
     half-dh tile lands already "folded" into the [64-partition, 2-ktile]
     layout the DoubleRow scores need — the fold is a pure host-side
     permutation). x DMAs are pair-merged via a host layout and issued
     round-robin from 4 engines (dma_start costs the issuer ~500ns).
  2. v projection bf16 -> PE-transpose -> seq-major v_sm, streamed
     concurrently with early attention q-blocks.
  3. attention, qb outer / head inner; per (head, q-block of 1024):
       S.T tile [k:128, q:1024] = DoubleRow(kf, qf)
       P = exp(S.T * 2^-15)   ACT, PSUM -> SBUF bf16 (the only ACT work;
         ACT is the attention-phase critical path at ~1.04us/tile)
       denominator: DVE pair-add tree over the 32 P tiles (bf16 L0-L2,
         f32 L3-L4), then r = ones.T @ acc via f32r matmuls (1 cyc/row),
         reciprocal f32, broadcast via f32r matmul; evict oT2 = po*rbin
         on Pool.
       O.T[dh, q] += v_sm_kb @ P   (bf16)
     o-proj work for q-block N-1 is interleaved INTO q-block N's kb loop
     (one seq-tile per 4 kb) so its PE burst doesn't starve ACT.
Host: out = sum_c(partial_c) + o_b, partials in bf16.
"""

import math
import sys

sys.path.insert(0, "/opt/trn_rl_repo")

import numpy as np
import ml_dtypes

HIDDEN = 2048
HEADS = 16
DH = 128
S = 4096
NCORES = 8
HPC = HEADS // NCORES  # heads per core = 2
DPC = HPC * DH         # head dims per core = 256
QB = 1024              # q-block size in attention
BF16 = ml_dtypes.bfloat16
FP8 = ml_dtypes.float8_e4m3
SQ = 2.0 ** 9          # host scale on Wq (q stored as fp8 of q*SQ)
SK = 2.0 ** 6          # host scale on Wk
EXPSCALE = 1.0 / (SQ * SK)

_cache = {}


def build_nc(debug=False):
    import concourse.bacc as bacc
    import concourse.mybir as mybir
    import concourse.tile as tile
    from concourse import bass_isa
    from concourse.masks import make_identity

    dt = mybir.dt
    AF = mybir.ActivationFunctionType
    ALU = mybir.AluOpType
    DR = mybir.MatmulPerfMode.DoubleRow

    nc = bacc.Bacc(None, target_bir_lowering=False, debug=debug)
    # paired layouts: row (i2*128+p), col (chunk*1024 + t*512 + c) holds
    # xT[(i2*2+t)*128 + p, chunk*512 + c]
    xt8_d = nc.dram_tensor("xt8", [HIDDEN // 2, 2 * S], dt.float8e4,
                           kind="ExternalInput")
    xtb_d = nc.dram_tensor("xtb", [HIDDEN // 2, 2 * S], dt.bfloat16,
                           kind="ExternalInput")
    w8_ds = {
        p: nc.dram_tensor(f"w8{p}", [128, 8 * 2 * 256], dt.float8e4,
                          kind="ExternalInput")
        for p in "qk"
    }
    wv_d = nc.dram_tensor("wv", [128, 16 * 256], dt.bfloat16, kind="ExternalInput")
    wo_d = nc.dram_tensor("wo", [128, HPC * HIDDEN], dt.bfloat16,
                          kind="ExternalInput")
    out_d = nc.dram_tensor("out", [S, HIDDEN], dt.bfloat16,
                           kind="ExternalOutput")

    with tile.TileContext(nc) as tc:
        with tc.tile_pool(name="persist", bufs=1) as pp, \
             tc.tile_pool(name="xstr", bufs=12) as xp, \
             tc.tile_pool(name="pt", bufs=12) as ptp, \
             tc.tile_pool(name="trb", bufs=3) as trb, \
             tc.tile_pool(name="trf", bufs=3) as trf, \
             tc.tile_pool(name="rnorm", bufs=1) as rnp, \
             tc.tile_pool(name="outst", bufs=4) as osp, \
             tc.tile_pool(name="qkv_ps", bufs=2, space="PSUM") as qps, \
             tc.tile_pool(name="ps_s", bufs=2, space="PSUM") as pss, \
             tc.tile_pool(name="ps_o", bufs=1, space="PSUM") as pso:
            # ---- persistent tiles ----
            w8 = {}
            for p in "qk":
                w8[p] = pp.tile([128, 8, 2, 256], dt.float8e4, tag=f"w8{p}",
                                name=f"w8{p}")
                nc.sync.dma_start(out=w8[p][:], in_=w8_ds[p][:])
            wv_s = pp.tile([128, 16, 256], dt.bfloat16, tag="wv", name="wv_s")
            nc.sync.dma_start(out=wv_s[:], in_=wv_d[:])
            wo_s = pp.tile([128, HPC, HIDDEN], dt.bfloat16, tag="wo", name="wo_s")
            nc.sync.dma_start(out=wo_s[:], in_=wo_d[:])
            qf = pp.tile([128, 2, S], dt.float8e4, tag="qf", name="qf")
            kf = pp.tile([128, 2, S], dt.float8e4, tag="kf", name="kf")
            vT = {h: pp.tile([128, S], dt.bfloat16, tag=f"vT{h}", name=f"vT{h}")
                  for h in range(HPC)}
            v_sm = {h: pp.tile([128, S], dt.bfloat16, tag=f"vsm{h}", name=f"vsm{h}")
                    for h in range(HPC)}
            oT2 = pp.tile([128, HPC, S], dt.bfloat16, tag="oT2", name="oT2")
            ident = pp.tile([128, 128], dt.bfloat16, tag="ident", name="ident")
            make_identity(nc, ident[:])
            dma_engs = [nc.sync, nc.scalar, nc.gpsimd]
            dma_rr = [0]

            def dma(out, in_, engs=None):
                engs = engs or dma_engs
                eng = engs[dma_rr[0] % len(engs)]
                dma_rr[0] += 1
                eng.dma_start(out=out, in_=in_)

            # ---- Stage 1a: q,k projections fused, fp8 DoubleRow ----
            # x8 streamed once per chunk; q psum in pss (double-buffered
            # [128,1024], d-halves in columns), k psum in pso.
            for chunk in range(8):
                base = chunk * 512
                ps_q = pss.tile([128, 1024], dt.float32, tag="pss",
                                name=f"psq_{chunk}")
                ps_k = pso.tile([128, 1024], dt.float32, tag="pso",
                                name=f"psk_{chunk}")
                x8ts = []
                for i2 in range(8):
                    x8t = xp.tile([128, 2, 512], dt.float8e4, tag="x8",
                                  name=f"x8_{chunk}_{i2}")
                    if chunk == 0:
                        for t in range(2):
                            dma(x8t[:, t, :],
                                xt8_d[i2 * 128:(i2 + 1) * 128,
                                      t * 512:(t + 1) * 512])
                    else:
                        dma(x8t[:], xt8_d[i2 * 128:(i2 + 1) * 128,
                                          chunk * 1024:(chunk + 1) * 1024])
                    x8ts.append(x8t)
                for i2 in range(8):
                    for d in range(2):
                        nc.tensor.matmul(
                            ps_q[:, d * 512:(d + 1) * 512],
                            w8["q"][:, i2, :, d * 128:(d + 1) * 128],
                            x8ts[i2][:],
                            start=(i2 == 0),
                            stop=(i2 == 7),
                            perf_mode=DR,
                            skip_group_check=True,
                        )
                for i2 in range(8):
                    for d in range(2):
                        nc.tensor.matmul(
                            ps_k[:, d * 512:(d + 1) * 512],
                            w8["k"][:, i2, :, d * 128:(d + 1) * 128],
                            x8ts[i2][:],
                            start=(i2 == 0),
                            stop=(i2 == 7),
                            perf_mode=DR,
                            skip_group_check=True,
                        )
                for d in range(2):
                    nc.vector.tensor_copy(qf[:, d, base:base + 512],
                                          ps_q[:, d * 512:(d + 1) * 512])
                    nc.vector.tensor_copy(kf[:, d, base:base + 512],
                                          ps_k[:, d * 512:(d + 1) * 512])

            # ---- Stage 1b: v projection (bf16) + transpose to seq-major.
            # Chunks 0-3 inline (qps/pso psums only -- pss stays free so
            # attention's score psums are never blocked); chunks 4-7 are
            # emitted inside the first attention block's kb loop.
            vdma_tiles = {}

            def emit_vchunk_dma(chunk, engs):
                tiles = []
                for i2 in range(8):
                    xbt = xp.tile([128, 2, 512], dt.bfloat16, tag="xb",
                                  name=f"xb_{chunk}_{i2}")
                    dma(xbt[:], xtb_d[i2 * 128:(i2 + 1) * 128,
                                      chunk * 1024:(chunk + 1) * 1024],
                        engs=engs)
                    tiles.append(xbt)
                vdma_tiles[chunk] = tiles

            def emit_vchunk_mm(chunk, tpools):
                base = chunk * 512
                ps_h = [qps.tile([128, 512], dt.float32, tag="rps",
                                 name=f"psv_{chunk}_{h}")
                        for h in range(HPC)]
                for i2 in range(8):
                    xbt = vdma_tiles[chunk][i2]
                    for t in range(2):
                        for h in range(HPC):
                            nc.tensor.matmul(
                                ps_h[h][:],
                                wv_s[:, i2 * 2 + t, h * 128:(h + 1) * 128],
                                xbt[:, t, :],
                                start=(i2 == 0 and t == 0),
                                stop=(i2 == 7 and t == 1),
                            )
                for h in range(HPC):
                    nc.vector.tensor_copy(vT[h][:, base:base + 512], ps_h[h][:])
                for h in range(HPC):
                    for jj in range(4):
                        j = chunk * 4 + jj
                        tp_t = tpools[jj % len(tpools)].tile(
                            [128, 128], dt.bfloat16, tag="rps" if True else "",
                            name=f"vt_{h}_{j}")
                        nc.tensor.transpose(
                            tp_t[:], vT[h][:, j * 128:(j + 1) * 128], ident[:]
                        )
                        nc.vector.tensor_copy(
                            v_sm[h][:, j * 128:(j + 1) * 128], tp_t[:]
                        )

            for chunk in range(4):
                emit_vchunk_dma(chunk, [nc.sync, nc.gpsimd])
                emit_vchunk_mm(chunk, [qps])
            # prefetch xb for chunks 4,5 (6,7 follow inside attention)
            emit_vchunk_dma(4, [nc.sync, nc.gpsimd])
            emit_vchunk_dma(5, [nc.sync, nc.gpsimd])

            # ---- Stage 2: attention; o-proj of earlier q-blocks
            # interleaved one (t,nb) tile per kb ----
            oproj_work = []  # (qb, t, nb)

            def emit_oproj(t, nb, drain_i=None):
                if drain_i is not None and drain_i % 2 == 1:
                    ps = pss.tile([128, 512], dt.float32, tag="pss",
                                  name=f"ops_{t}_{nb}")
                else:
                    ps = qps.tile([128, 512], dt.float32, tag="rps",
                                  name=f"ops_{t}_{nb}")
                for h in range(HPC):
                    nc.tensor.matmul(
                        ps[:],
                        oT2[:, h, t * 128:(t + 1) * 128],
                        wo_s[:, h, nb * 512:(nb + 1) * 512],
                        start=(h == 0),
                        stop=(h == HPC - 1),
                    )
                ot_ = osp.tile([128, 512], dt.bfloat16, tag="outst",
                               name=f"ot_{t}_{nb}")
                nc.vector.tensor_copy(ot_[:], ps[:])
                dma(out_d[t * 128:(t + 1) * 128, nb * 512:(nb + 1) * 512],
                    ot_[:], engs=[nc.sync, nc.gpsimd])


            def finish_block(qb, h, po, acc):
                # PE-free normalization: cross-partition sum on Pool,
                # reciprocal + scale on DVE.
                rsum = rnp.tile([128, QB], dt.float32, tag="rsum",
                                name=f"rsum_{qb}_{h}")
                nc.gpsimd.partition_all_reduce(rsum[:], acc[:], 128,
                                               bass_isa.ReduceOp.add)
                rinv = rnp.tile([128, QB], dt.float32, tag="rinv",
                                name=f"rinv_{qb}_{h}")
                nc.vector.reciprocal(rinv[:], rsum[:])
                nc.vector.tensor_tensor(
                    oT2[:, h, qb * QB:(qb + 1) * QB], po[:], rinv[:], ALU.mult,
                )
                if h == HPC - 1:
                    for t in range(qb * (QB // 128), (qb + 1) * (QB // 128)):
                        for nb in range(HIDDEN // 512):
                            oproj_work.append((t, nb))

            for qb in range(S // QB):
                for h in range(HPC):
                    first_block = (qb == 0 and h == 0)
                    po = pso.tile([128, QB], dt.float32, tag="pso",
                                  name=f"po_{qb}_{h}")
                    s1 = []
                    s2 = []
                    s3 = []
                    s4 = []
                    acc = None
                    for kb in range(S // 128):
                        ps = pss.tile([128, QB], dt.float32, tag="pss",
                                      name=f"ps_{qb}_{h}_{kb}")
                        for j in range(QB // 512):
                            nc.tensor.matmul(
                                ps[:, j * 512:(j + 1) * 512],
                                kf[h * 64:(h + 1) * 64, :, kb * 128:(kb + 1) * 128],
                                qf[h * 64:(h + 1) * 64, :,
                                   qb * QB + j * 512:qb * QB + (j + 1) * 512],
                                start=True,
                                stop=True,
                                perf_mode=DR,
                            )
                        pt = ptp.tile([128, QB], dt.bfloat16, tag="pt",
                                      name=f"pt_{qb}_{h}_{kb}")
                        nc.scalar.activation(pt[:], ps[:], AF.Exp, scale=EXPSCALE)
                        for j in range(QB // 512):
                            nc.tensor.matmul(
                                po[:, j * 512:(j + 1) * 512],
                                v_sm[h][:, kb * 128:(kb + 1) * 128],
                                pt[:, j * 512:(j + 1) * 512],
                                start=(kb == 0),
                                stop=(kb == 31),
                                skip_group_check=True,
                            )
                        # first block: stream v chunks 4-7 through spare slots
                        if first_block:
                            if kb == 5:
                                emit_vchunk_dma(6, [nc.sync, nc.gpsimd])
                            elif kb == 11:
                                emit_vchunk_dma(7, [nc.sync, nc.gpsimd])
                            if kb in (3, 9, 15, 21):
                                emit_vchunk_mm(4 + (kb - 3) // 6, [qps])
                        if kb % 2 == 1 and kb >= 5 and oproj_work:
                            emit_oproj(*oproj_work.pop(0))
                        if kb in (21, 23, 25, 27) and oproj_work:
                            emit_oproj(*oproj_work.pop(0))
                        # denominator pair-add tree on DVE
                        s1.append(pt)
                        if len(s1) == 2:
                            a, b = s1
                            o = trb.tile([128, QB], dt.bfloat16, tag="s1",
                                         name=f"s1_{qb}_{h}_{kb}")
                            nc.vector.tensor_tensor(o[:], a[:], b[:], ALU.add)
                            s1 = []
                            s2.append(o)
                        if len(s2) == 2:
                            a, b = s2
                            o = trb.tile([128, QB], dt.bfloat16, tag="s2",
                                         name=f"s2_{qb}_{h}_{kb}")
                            nc.vector.tensor_tensor(o[:], a[:], b[:], ALU.add)
                            s2 = []
                            s3.append(o)
                        if len(s3) == 2:
                            a, b = s3
                            s3 = []
                            acc2 = trf.tile([128, QB], dt.float32, tag="trf",
                                            name=f"acc_{qb}_{h}_{kb}")
                            if acc is None:
                                nc.vector.tensor_tensor(acc2[:], a[:], b[:],
                                                        ALU.add)
                            else:
                                o = trb.tile([128, QB], dt.bfloat16, tag="s3",
                                             name=f"s3_{qb}_{h}_{kb}")
                                nc.vector.tensor_tensor(o[:], a[:], b[:],
                                                        ALU.add)
                                nc.vector.tensor_tensor(acc2[:], acc[:], o[:],
                                                        ALU.add)
                            acc = acc2
                    finish_block(qb, h, po, acc)
            # drain remaining o-proj work (last q-block) with a deeper
            # psum rotation -- the score pool is idle by now
            for di, item in enumerate(oproj_work):
                emit_oproj(*item, drain_i=di)
    nc.finalize()
    return nc


def host_prep(hidden_states, q_V, q_U, k_V, k_U, v_V, v_U, o_W):
    """Build per-core input maps (host-side sharding + layout)."""
    x = np.asarray(hidden_states, np.float32).reshape(S, HIDDEN)
    xT = np.ascontiguousarray(x.T)

    def paired(xt):  # [HIDDEN, S] -> [HIDDEN/2, 2S] pair-merged DMA layout
        A = xt.reshape(8, 2, 128, 8, 512)        # [i2, t, p, chunk, c]
        A = A.transpose(0, 2, 3, 1, 4)           # [i2, p, chunk, t, c]
        return np.ascontiguousarray(A.reshape(HIDDEN // 2, 2 * S))

    xT8 = paired(xT).astype(FP8)
    xTb = paired(xT).astype(BF16)
    Wq = (np.asarray(q_U, np.float32) @ np.asarray(q_V, np.float32)) \
        / math.sqrt(DH) * SQ
    Wk = np.asarray(k_U, np.float32) @ np.asarray(k_V, np.float32) * SK
    Wv = np.asarray(v_U, np.float32) @ np.asarray(v_V, np.float32)
    oW = np.asarray(o_W, np.float32)

    def w8_image(WT):
        # [HIDDEN, DPC] -> [128, 8*2*2*128] fp8 image with folded column
        # order: free idx = i2*512 + t*256 + d*128 + h*64 + j, selecting
        # WT[(i2*2+t)*128 + p, h*128 + d*64 + j].
        A = WT.reshape(8, 2, 128, HPC, 2, 64)   # [i2, t, p, h, d, j]
        A = A.transpose(2, 0, 1, 4, 3, 5)       # [p, i2, t, d, h, j]
        return np.ascontiguousarray(A.reshape(128, 8 * 2 * 2 * 128)).astype(FP8)

    def wv_image(WT):  # [HIDDEN, DPC] -> [128, 16*DPC] sbuf image
        return np.ascontiguousarray(
            WT.reshape(16, 128, DPC).transpose(1, 0, 2).reshape(128, 16 * DPC)
        ).astype(BF16)

    def wo_image(oWcT):  # [DPC, HIDDEN] -> [128, HPC*HIDDEN]
        return np.ascontiguousarray(
            oWcT.reshape(HPC, 128, HIDDEN).transpose(1, 0, 2).reshape(128, HPC * HIDDEN)
        ).astype(BF16)

    in_maps = []
    for c in range(NCORES):
        sl = slice(c * DPC, (c + 1) * DPC)
        in_maps.append({
            "xt8": xT8,
            "xtb": xTb,
            "w8q": w8_image(np.ascontiguousarray(Wq[sl, :].T)),
            "w8k": w8_image(np.ascontiguousarray(Wk[sl, :].T)),
            "wv": wv_image(np.ascontiguousarray(Wv[sl, :].T)),
            "wo": wo_image(np.ascontiguousarray(oW[:, sl].T)),
        })
    return in_maps


def run(inputs, trace=False, tmpdir=None):
    from concourse.bass_utils import run_bass_kernel_spmd

    if "nc" not in _cache:
        _cache["nc"] = build_nc()
    nc = _cache["nc"]
    in_maps = host_prep(
        inputs["hidden_states"], inputs["q_V"], inputs["q_U"], inputs["k_V"],
        inputs["k_U"], inputs["v_V"], inputs["v_U"], inputs["o_W"],
    )
    res = run_bass_kernel_spmd(
        nc, in_maps, core_ids=list(range(NCORES)), trace=trace, tmpdir=tmpdir
    )
    acc = np.zeros((S, HIDDEN), np.float64)
    for c in range(NCORES):
        acc += res.results[c]["out"].astype(np.float64)
    out = (acc + np.asarray(inputs["o_b"], np.float64)[None, :]).astype(np.float32)
    return out.reshape(1, S, HIDDEN), res


def kernel(**inputs) -> np.ndarray:
    out, _ = run(inputs, trace=False)
    return out
